# revision 27
# baseline (speedup 1.0000x reference)
"""DigitCapsule dynamic-routing kernel for 8 TRN2 NeuronCores.

Strategy: routing is independent per output capsule c, so shard C=64
capsules 8-ways; zero collectives, identical SPMD program per core.

Per core (B=64, R=2048, I=8, CL=8, O=16; K-dim (r,i) = 16384 = 128 k-tiles,
16 h-blocks of 8 tiles, 4 groups of 4 h-blocks):
  pass 0 (flipped): s0[b,(o,c)] = sum_t xt_t^T @ Wr_t      (c_ij uniform)
  iter 1,2:
    G_h[(q,i),(lo,o,c)] = xn^T @ V  per h-block   (K=b=64)
    P_h = G_h (.) Wr_h  ->  BD band-matmul sums i -> bands (32j+q)
    ored sums o -> ab_band[128,(g,lo,c)] ; bstate += ab/B ; w = exp(b)
    wrep_h via RepM matmul from band rows; WW_h = Wr_h (.) w
    N^T[(o,c),b] = sum_t WW_t^T @ XT_t ; Z[c] = sum_r w ; s = N^T/Z
    v = squash(s) = s*|s|/(1+s^2)   (exact; eps only guards 0/0)
  out = v^T -> [b,(o,c)]

DMA: two HW queues (sync: per-chunk wk+xt 768KB blocks; scalar: consts
then per-chunk xn) so input streams at full per-core HBM bandwidth with
few trigger instructions.
"""

import os
import sys

for _p in ("/opt/trn_rl_repo", "/root/.axon_site/_ro/trn_rl_repo"):
    if os.path.isdir(_p) and _p not in sys.path:
        sys.path.insert(0, _p)

from contextlib import ExitStack

import numpy as np

import concourse.bass as bass
import concourse.bacc as bacc
from concourse import mybir
from concourse.bass_utils import run_bass_kernel_spmd
from concourse.tile import TileContext

B, R, C, O, I = 64, 2048, 64, 16, 8
N_CORES = 8
CL = C // N_CORES            # capsules per core = 8
F = CL * O                   # free (c,o) = 128
NT = R // 16                 # 128 k-tiles; tile t = routes [16t,16t+16)
NH = 16                      # h-blocks (8 k-tiles each)
BLK = NT // NH               # 8
NG = 4                       # groups of 4 h-blocks
COMPUTE = "bf16"

# cst32 cols: ident [0:128), RepC [128:256), band mask [256:257)
CW32 = 257
# cstb cols: BDF4 [0:512), RepM [512:640), RepM-hi for band 3 [640:768)
CWB = 768


def _consts_np():
    p = np.arange(128)
    c32 = np.zeros((128, CW32), dtype=np.float32)
    c32[p, p] = 1.0                       # ident
    for c in range(8):                    # RepC[c, 128+o*8+c] = 1
        c32[c, 128 + c:256:8] = 1.0
    c32[p % 32 < 16, 256] = 1.0           # mask: 1 on band rows 32j+q, q<16
    cb = np.zeros((128, CWB), dtype=np.float32)
    for j in range(4):                    # BDF4_j[p, 32j + p//8] = 1 (slice j)
        cb[p, 128 * j + 32 * j + p // 8] = 1.0
    # RepM replicated at each 32-row band: rows 32j+q (q<16) hold pattern(q)
    for j in range(4):
        for q in range(16):
            cb[32 * j + q, 512 + q * 8:512 + q * 8 + 8] = 1.0
    # RepM-hi: for band 3 (abs rows 96+q) read via a base-64 K=64 matmul
    for q in range(16):
        cb[96 + q, 640 + q * 8:640 + q * 8 + 8] = 1.0
    return c32, cb


def _squash(nc, pool, s_sb, Pdim, Nfree):
    """v = s*|s|/(1+s^2), elementwise f32 on [Pdim, Nfree]; all DVE."""
    f32 = mybir.dt.float32
    sq = pool.tile([Pdim, Nfree], f32, tag="sq", name="sq")
    ab_ = pool.tile([Pdim, Nfree], f32, tag="as", name="as")
    num = pool.tile([Pdim, Nfree], f32, tag="num", name="num")
    den = pool.tile([Pdim, Nfree], f32, tag="den", name="den")
    rd = pool.tile([Pdim, Nfree], f32, tag="rd", name="rd")
    v = pool.tile([Pdim, Nfree], f32, tag="v", name="v")
    nc.vector.scalar_tensor_tensor(ab_, s_sb, -1.0, s_sb,
                                   op0=mybir.AluOpType.mult,
                                   op1=mybir.AluOpType.max)
    nc.vector.tensor_mul(sq, s_sb, s_sb)
    nc.vector.tensor_mul(num, s_sb, ab_)
    nc.vector.tensor_scalar_add(den, sq, 1.0)
    nc.vector.reciprocal_approx_fast(rd, den)
    nc.vector.tensor_mul(v, num, rd)
    return v


def build_bass():
    f32 = mybir.dt.float32
    cdt = mybir.dt.bfloat16

    nc = bacc.Bacc()
    # wxt: 8 chunks of [wk 2048 | xt 1024 | xn 2048] columns
    wxt_d = nc.declare_dram_parameter("wxt", [128, 8 * 5120], cdt, isOutput=False)
    c32_d = nc.declare_dram_parameter("cst32", [128, CW32], f32, isOutput=False)
    cb_d = nc.declare_dram_parameter("cstb", [128, CWB], cdt, isOutput=False)
    out_d = nc.declare_dram_parameter("out", [B, F], f32, isOutput=True)

    with TileContext(nc) as tc, ExitStack() as ctx:
        big = ctx.enter_context(tc.tile_pool(name="big", bufs=1))
        small = ctx.enter_context(tc.tile_pool(name="small", bufs=3))
        sq_pool = ctx.enter_context(tc.tile_pool(name="sqp", bufs=2))
        ppool = ctx.enter_context(tc.tile_pool(name="ppool", bufs=3))
        wwpool = ctx.enter_context(tc.tile_pool(name="wwpool", bufs=3))
        wrpool = ctx.enter_context(tc.tile_pool(name="wrpool", bufs=3))
        ps_acc = ctx.enter_context(tc.tile_pool(name="ps_acc", bufs=1, space="PSUM"))
        ps_g = ctx.enter_context(tc.tile_pool(name="ps_g", bufs=2, space="PSUM"))
        ps_bd = ctx.enter_context(tc.tile_pool(name="ps_bd", bufs=2, space="PSUM"))
        ps_wr = ctx.enter_context(tc.tile_pool(name="ps_wr", bufs=1, space="PSUM"))

        # ---- input DMA: sync queue = wk+xt big blocks; scalar queue = consts+xn
        wxt = [big.tile([128, 5120], cdt, tag=f"wxt{h}", name=f"wxt{h}")
               for h in range(8)]
        c32 = big.tile([128, CW32], f32, tag="c32", name="c32")
        cb = big.tile([128, CWB], cdt, tag="cb", name="cb")
        nc.scalar.dma_start(out=c32, in_=c32_d[:])
        nc.scalar.dma_start(out=cb, in_=cb_d[:])
        # strict priority on the sync queue: all wk+xt (pass0-critical)
        # first, then xn (first needed at iter-1 start)
        for h in range(8):
            nc.sync.dma_start(out=wxt[h][:, 0:3072],
                              in_=wxt_d[:, h * 5120:h * 5120 + 3072])
        for h in range(8):
            nc.sync.dma_start(out=wxt[h][0:64, 3072:5120],
                              in_=wxt_d[0:64, h * 5120 + 3072:(h + 1) * 5120])

        ident = c32[:, 0:128]
        RepC = c32[0:8, 128:256]
        bandmask = c32[:, 256:257]
        BDF4c = cb[:, 0:512]

        def wk_tile(t):
            h, lo = t // 16, t % 16
            return wxt[h][:, lo * 128:(lo + 1) * 128]

        def xt_tile(t):
            h, lo = t // 16, t % 16
            return wxt[h][:, 2048 + lo * 64:2048 + (lo + 1) * 64]

        def xn_col(t):
            h, lo = t // 16, t % 16
            return wxt[h][0:64, 3072 + lo * 128:3072 + (lo + 1) * 128]

        # ---- pass 0: s0^T[(o,c),b] = sum_t wk_t^T @ xt_t (LD-hidden) ----
        ps_s0 = ps_acc.tile([128, 64], f32, tag="acc", name="acc0")
        for t in range(NT):
            nc.tensor.matmul(ps_s0, lhsT=wk_tile(t), rhs=xt_tile(t),
                             start=(t == 0), stop=(t == NT - 1))
        s0 = small.tile([128, 64], f32, tag="s0", name="s0")
        nc.vector.tensor_scalar_mul(s0, ps_s0, 1.0 / R)
        v0 = _squash(nc, sq_pool, s0, 128, 64)
        ps_t0 = ps_acc.tile([64, 128], f32, tag="acc", name="vt0")
        nc.tensor.transpose(ps_t0, v0, ident)
        V2 = small.tile([64, 128], cdt, tag="v2", name="v2", bufs=2)
        nc.vector.tensor_copy(V2, ps_t0)

        # band-layout routing state: rows (32j+q), cols (g, lo, c)
        bstate = small.tile([128, 256], f32, tag="bstate", name="bstate", bufs=1)
        nc.vector.memset(bstate, 0.0)

        for it in (1, 2):
            wexp = small.tile([128, 256], cdt, tag="wexp", name="wexp", bufs=2)
            abb = small.tile([128, 256], f32, tag="abb", name="abb", bufs=2)
            ps_n = ps_acc.tile([128, 64], f32, tag="acc", name="accn")
            psb = [None] * NG
            wrs = [None] * NH

            def emit_G(h):
                psg = ps_g.tile([128, BLK * 128], f32, tag="g", name="g")
                for lo in range(BLK):
                    t = h * BLK + lo
                    nc.tensor.matmul(
                        psg[:, lo * 128:(lo + 1) * 128],
                        lhsT=xn_col(t), rhs=V2, start=True, stop=True)
                return psg

            psgs = {}

            def emit_apath_tail(h, psg):
                g, j = h // 4, h % 4
                # psum f32 -> sbuf bf16 cast: ACT (12) / Pool (4)
                Pg = ppool.tile([128, BLK * 128], cdt, tag="Pg", name="Pg")
                nc.scalar.activation(Pg, psg,
                                     mybir.ActivationFunctionType.Copy)
                P = ppool.tile([128, BLK * 128], cdt, tag="P", name="P")
                wkh = wxt[h // 2][:, 0:2048].rearrange("p (u f) -> p u f", f=128)
                nc.vector.tensor_mul(
                    P.rearrange("p (u f) -> p u f", f=128),
                    Pg.rearrange("p (u f) -> p u f", f=128),
                    wkh[:, (h % 2) * BLK:(h % 2) * BLK + BLK, :],
                )
                if j == 0:
                    psb[g] = ps_bd.tile([128, 512], f32, tag="bd", name="bd")
                # accumulate the two o-halves into the same psum columns:
                # psb cols (lo, o', c) with o' in [0,8)
                for oh in range(2):
                    rhs = bass.AP(tensor=P.tensor,
                                  offset=P[:, 64 * oh:64 * oh + 1].offset,
                                  ap=[P.ap[0], [128, 8], [8, 8], [1, 8]])
                    nc.tensor.matmul(
                        psb[g],
                        lhsT=BDF4c[:, 128 * j:128 * (j + 1)],
                        rhs=rhs,
                        start=(j == 0 and oh == 0), stop=(j == 3 and oh == 1),
                    )

            def emit_group_close(g):
                # o-reduce on Pool -> ab band slice; bstate += ab/B; exp
                gs = slice(g * 64, (g + 1) * 64)
                nc.vector.tensor_reduce(
                    abb[:, gs].rearrange("p (l c) -> p l c", c=8),
                    bass.AP(tensor=psb[g].tensor, offset=psb[g].offset,
                            ap=[psb[g].ap[0], [64, 8], [1, 8], [8, 8]]),
                    axis=mybir.AxisListType.X,
                    op=mybir.AluOpType.add,
                )
                nc.vector.scalar_tensor_tensor(
                    bstate[:, gs], abb[:, gs], 1.0 / B, bstate[:, gs],
                    op0=mybir.AluOpType.mult, op1=mybir.AluOpType.add)
                nc.scalar.activation(wexp[:, gs], bstate[:, gs],
                                     mybir.ActivationFunctionType.Exp)

            def emit_bpath(h):
                g, j = h // 4, h % 4
                # wrep_h[(q,i),(lo,c)] from band rows 32j..32j+16
                ps_w = ps_wr.tile([128, 64], f32, tag="wr", name="wrps")
                gs = slice(g * 64, (g + 1) * 64)
                if j < 3:
                    nc.tensor.matmul(ps_w, lhsT=cb[32 * j:32 * j + 16, 512:640],
                                     rhs=wexp[32 * j:32 * j + 16, gs],
                                     start=True, stop=True)
                else:
                    nc.tensor.matmul(ps_w, lhsT=cb[64:128, 640:768],
                                     rhs=wexp[64:128, gs],
                                     start=True, stop=True)
                wr = wrpool.tile([128, 64], cdt, tag="wr", name="wrs")
                nc.scalar.activation(wr, ps_w,
                                     mybir.ActivationFunctionType.Copy)
                wrs[h] = wr
                # WW_h = wk_h (.) wrep (broadcast over o)
                ww = wwpool.tile([128, 8 * 128], cdt, tag="ww", name="ww")
                wkh = wxt[h // 2][:, (h % 2) * 1024:(h % 2) * 1024 + 1024]
                in1 = bass.AP(tensor=wr.tensor, offset=wr.offset,
                              ap=[wr.ap[0], [8, 8], [0, 16], [1, 8]])
                eng = nc.gpsimd if (h % 4 == 1 and h < 12) else nc.vector
                eng.tensor_tensor(
                    ww.rearrange("p (l o c) -> p l o c", o=16, c=8),
                    wkh.rearrange("p (l o c) -> p l o c", o=16, c=8),
                    in1, op=mybir.AluOpType.mult)
                return ww

            def emit_N(h, ww):
                for lo in range(BLK):
                    t = h * BLK + lo
                    nc.tensor.matmul(ps_n,
                                     lhsT=ww[:, lo * 128:(lo + 1) * 128],
                                     rhs=xt_tile(t),
                                     start=(t == 0), stop=(t == NT - 1))

            # software-pipelined emission: keep PE fed; N(h-1) MMs cover the
            # single-bank wrep WAR between consecutive wrep matmuls
            wws = {}
            for step in range(NH + 2):
                if step < NH:
                    psgs[step] = emit_G(step)
                h2 = step - 2
                if h2 >= 0:
                    emit_apath_tail(h2, psgs.pop(h2))
                    if h2 % 4 == 3:
                        g = h2 // 4
                        emit_group_close(g)
                        for hh in range(4 * g, 4 * g + 4):
                            wws[hh] = emit_bpath(hh)
                            if hh - 1 in wws:
                                emit_N(hh - 1, wws.pop(hh - 1))
            emit_N(NH - 1, wws.pop(NH - 1))

            # Z[c] = sum_r w; rz = 1/Z; rzrep per (o,c) partition
            wsum = small.tile([128, 8], f32, tag="wsum", name="wsum")
            nc.vector.tensor_reduce(
                wsum,
                bass.AP(tensor=wexp.tensor, offset=wexp.offset,
                        ap=[wexp.ap[0], [1, 8], [8, 32]]),
                axis=mybir.AxisListType.X, op=mybir.AluOpType.add)
            ps_z = ps_wr.tile([8, 1], f32, tag="wr", name="zps")
            nc.tensor.matmul(ps_z, lhsT=wsum, rhs=bandmask, start=True, stop=True)
            rz = small.tile([8, 1], f32, tag="rz", name="rz")
            nc.vector.reciprocal_approx_fast(rz, ps_z)
            ps_rz = ps_wr.tile([128, 1], f32, tag="wr", name="rzps")
            nc.tensor.matmul(ps_rz, lhsT=RepC, rhs=rz, start=True, stop=True)
            # s = N^T * (1/Z) read straight from psum ; v = squash(s)
            s_it = small.tile([128, 64], f32, tag="s", name="s")
            nc.vector.tensor_scalar_mul(s_it, ps_n, ps_rz)
            vT = _squash(nc, sq_pool, s_it, 128, 64)
            if it < 2:
                ps_t = ps_acc.tile([64, 128], f32, tag="acc", name="vt")
                nc.tensor.transpose(ps_t, vT, ident)
                V2 = small.tile([64, 128], cdt, tag="v2", name="v2", bufs=2)
                nc.vector.tensor_copy(V2, ps_t)

        # ---- output: out[b,(o,c)] = vT^T ----
        ps_o = ps_acc.tile([64, 128], f32, tag="acc", name="out_ps")
        nc.tensor.transpose(ps_o, vT, ident)
        out_sb = small.tile([64, 128], f32, tag="outsb", name="outsb")
        nc.vector.tensor_copy(out_sb, ps_o)
        nc.sync.dma_start(out=out_d[:], in_=out_sb)

    nc.finalize()
    return nc


def _host_prep(x, W):
    import ml_dtypes
    ct = ml_dtypes.bfloat16
    x = np.ascontiguousarray(x, dtype=np.float32)
    W = np.ascontiguousarray(W, dtype=np.float32)
    # xt[p=(q,i), t*64+b] = x[b, 16t+q, i]
    xt = x.reshape(B, NT, 16, I).transpose(2, 3, 1, 0).reshape(128, NT, 64)
    # xn[b, t*128 + q*8+i] = x[b, 16t+q, i]
    xn = x.reshape(B, NT, 128)
    c32, cbf = _consts_np()
    in_maps = []
    for k in range(N_CORES):
        Ws = W[:, k * CL:(k + 1) * CL]  # [R, 8, O, I]
        wk = (Ws.reshape(NT, 16, CL, O, I).transpose(1, 4, 0, 3, 2)
              .reshape(128, NT, 128))
        wxt = np.zeros((128, 8, 5120), dtype=np.float32)
        for h in range(8):
            wxt[:, h, 0:2048] = wk[:, 16 * h:16 * (h + 1), :].reshape(128, 2048)
            wxt[:, h, 2048:3072] = xt[:, 16 * h:16 * (h + 1), :].reshape(128, 1024)
            wxt[0:64, h, 3072:5120] = xn[:, 16 * h:16 * (h + 1), :].reshape(64, 2048)
        in_maps.append({
            "wxt": np.ascontiguousarray(wxt.reshape(128, 8 * 5120), dtype=ct),
            "cst32": c32,
            "cstb": cbf.astype(ct),
        })
    return in_maps


_CACHE = {}


def _get_nc():
    if "nc" not in _CACHE:
        _CACHE["nc"] = build_bass()
    return _CACHE["nc"]


def run(x, W, compute=COMPUTE, trace=False):
    nc = _get_nc()
    in_maps = _host_prep(x, W)
    res = run_bass_kernel_spmd(nc, in_maps, core_ids=list(range(N_CORES)),
                               trace=trace)
    outs = [np.asarray(res.results[k]["out"], dtype=np.float32)
            for k in range(N_CORES)]
    # out[b, (o, c)]: core k holds capsules [8k, 8k+8)
    v = np.concatenate(
        [o.reshape(B, O, CL).transpose(0, 2, 1) for o in outs], axis=1)
    return v[..., None], res


def kernel(x, W):
    v, _ = run(np.asarray(x), np.asarray(W))
    return v
